# revision 3
# baseline (speedup 1.0000x reference)
"""Causal self-attention (B=4, T=2048, C=1024, H=16, D=64) on 8 TRN2 NeuronCores.

Sharding: core c -> (batch b = c//2, head-group g = c%2 covering heads
8g..8g+8). Data-parallel over B, tensor-parallel over heads. The output
projection is computed per-core over its 512 channels; the two partial
products per batch are summed on the host (the "all-reduce"), where the
projection bias is also added.

Per-core kernel (single SPMD program, per-core data):
  Phase 1: qT = (x Wq + bq)^T and kT likewise, laid out [c'=512, T] (head-major
           on partitions: chunk m holds heads 2m, 2m+1 at partition offsets
           0/64); v in natural layout [T, c'] with an appended ones column per
           head (Vaug, M=65) for the softmax denominator.
  Phase 2: per head pair m, per 512-wide t-block: S^T[s,t] tiles via K=64
           matmuls packed two-heads-per-PE-array (tile_position from
           base_partition 0/64); exp on ScalarE (scale=1/8 folded) straight
           from PSUM into bf16 SBUF; causal masking by multiplying constant
           triangular masks on the diagonal s-chunks; AV matmuls with
           lhsT=Vaug -> unnormalized y^T (rows 0:64) and sumexp (row 64) in
           one accumulation; normalize with partition_broadcast + fast
           reciprocal.
  Phase 3: y_out^T = Wp_g^T @ yT accumulated over the core's 4 channel chunks,
           streamed to DRAM as bf16.

No max-subtraction in softmax: scores are O(1) here (exp is safe in fp32),
and exp(S)/sum(exp(S)) is mathematically identical to jax.nn.softmax.
"""

import numpy as np
import ml_dtypes

BF16 = ml_dtypes.bfloat16
F32 = np.float32

N_EMBD = 1024
N_HEAD = 16
HEAD_DIM = 64
B = 4
T = 2048
N_CORES = 8
HPC = 8          # heads per core
CPC = HPC * HEAD_DIM  # channels per core = 512
NKC = N_EMBD // 128   # contraction chunks over full embed = 8
NM = CPC // 128       # head-pair chunks per core = 4
NB = T // 512         # 512-wide t blocks = 4
NTT = T // 128        # 128-wide t tiles = 16

_BUILT = {}
LAST_RESULT = None  # BassKernelResults of the most recent run (for test harness)


def _build_nc():
    import concourse.bass as bass
    import concourse.mybir as mybir
    import concourse.tile as tile
    from concourse import bacc

    dt = mybir.dt
    AF = mybir.ActivationFunctionType
    ALU = mybir.AluOpType

    nc = bacc.Bacc(trn_type="TRN2", name="csa")

    # ---- DRAM I/O ----
    xT_d = nc.dram_tensor("xT", [N_EMBD, T], dt.bfloat16, kind="ExternalInput")
    wq_d = nc.dram_tensor("wq", [N_EMBD, CPC], dt.bfloat16, kind="ExternalInput")
    wk_d = nc.dram_tensor("wk", [N_EMBD, CPC], dt.bfloat16, kind="ExternalInput")
    wv_d = nc.dram_tensor("wv", [N_EMBD, CPC], dt.bfloat16, kind="ExternalInput")
    wp_d = nc.dram_tensor("wp", [CPC, N_EMBD], dt.bfloat16, kind="ExternalInput")
    bq_d = nc.dram_tensor("bq_r", [128, NM], dt.float32, kind="ExternalInput")
    bk_d = nc.dram_tensor("bk_r", [128, NM], dt.float32, kind="ExternalInput")
    bv_d = nc.dram_tensor("bv_bc", [128, CPC], dt.float32, kind="ExternalInput")
    mk_d = nc.dram_tensor("masks", [128, 4, 512], dt.bfloat16, kind="ExternalInput")

    kT_o = nc.dram_tensor("kT_out", [CPC, T], dt.bfloat16, kind="ExternalOutput")
    v_o = nc.dram_tensor("v_out", [T, CPC], dt.bfloat16, kind="ExternalOutput")
    yp_o = nc.dram_tensor("ypT_out", [N_EMBD, T], dt.bfloat16, kind="ExternalOutput")

    with tile.TileContext(nc) as tc:
        with tc.tile_pool(name="const", bufs=1) as cp, \
             tc.tile_pool(name="ex", bufs=3) as exp_pool, \
             tc.tile_pool(name="stage", bufs=4) as stage, \
             tc.tile_pool(name="small", bufs=2) as small:

            # ---- load constants / inputs ----
            xT_sb = cp.tile([128, NKC, T], dt.bfloat16, tag="xT")
            nc.sync.dma_start(xT_sb, xT_d.ap().rearrange("(kc p) t -> p kc t", p=128))
            wq_sb = cp.tile([128, NKC, CPC], dt.bfloat16, tag="wq")
            nc.sync.dma_start(wq_sb, wq_d.ap().rearrange("(kc p) m -> p kc m", p=128))
            wk_sb = cp.tile([128, NKC, CPC], dt.bfloat16, tag="wk")
            nc.sync.dma_start(wk_sb, wk_d.ap().rearrange("(kc p) m -> p kc m", p=128))
            wv_sb = cp.tile([128, NKC, CPC], dt.bfloat16, tag="wv")
            nc.sync.dma_start(wv_sb, wv_d.ap().rearrange("(kc p) m -> p kc m", p=128))
            wp_sb = cp.tile([128, NM, N_EMBD], dt.bfloat16, tag="wp")
            nc.sync.dma_start(wp_sb, wp_d.ap().rearrange("(kc p) m -> p kc m", p=128))
            bq_sb = cp.tile([128, NM], dt.float32, tag="bq")
            nc.sync.dma_start(bq_sb, bq_d.ap())
            bk_sb = cp.tile([128, NM], dt.float32, tag="bk")
            nc.sync.dma_start(bk_sb, bk_d.ap())
            bv_sb = cp.tile([128, CPC], dt.float32, tag="bv")
            nc.sync.dma_start(bv_sb, bv_d.ap())
            mk_sb = cp.tile([128, 4, 512], dt.bfloat16, tag="mk")
            nc.sync.dma_start(mk_sb, mk_d.ap())

            qT_sb = cp.tile([128, NM, T], dt.bfloat16, tag="qT")
            kT_sb = cp.tile([128, NM, T], dt.bfloat16, tag="kT")
            # Vaug: [t%128, tt, head, 65] (col 64 = ones)
            v_sb = cp.tile([128, NTT, HPC, 65], dt.bfloat16, tag="v")
            yT_sb = cp.tile([128, NM, T], dt.bfloat16, tag="yT")

            nc.gpsimd.memset(v_sb[:, :, :, 64:65], 1.0)

            # ---- Phase 1: QKV projections ----
            with tc.tile_pool(name="qkps", bufs=6, space="PSUM") as qkps:
                for (w_sb, bias_sb, out_sb) in ((wq_sb, bq_sb, qT_sb),
                                                (wk_sb, bk_sb, kT_sb)):
                    for m in range(NM):
                        for bb in range(NB):
                            ps = qkps.tile([128, 512], dt.float32, tag="qk")
                            for kc in range(NKC):
                                nc.tensor.matmul(
                                    ps,
                                    lhsT=w_sb[:, kc, 128 * m:128 * m + 128],
                                    rhs=xT_sb[:, kc, 512 * bb:512 * bb + 512],
                                    start=(kc == 0), stop=(kc == NKC - 1))
                            nc.vector.tensor_scalar(
                                out=out_sb[:, m, 512 * bb:512 * bb + 512],
                                in0=ps, scalar1=bias_sb[:, m:m + 1], scalar2=None,
                                op0=ALU.add)
                for tt in range(NTT):
                    ps = qkps.tile([128, 512], dt.float32, tag="qk")
                    for kc in range(NKC):
                        nc.tensor.matmul(
                            ps,
                            lhsT=xT_sb[:, kc, 128 * tt:128 * tt + 128],
                            rhs=wv_sb[:, kc, :],
                            start=(kc == 0), stop=(kc == NKC - 1))
                    nc.vector.tensor_tensor(
                        out=v_sb[:, tt, :, 0:64],
                        in0=ps.rearrange("p (h d) -> p h d", h=HPC),
                        in1=bv_sb.rearrange("p (h d) -> p h d", h=HPC),
                        op=ALU.add)

            # stream k / v to DRAM (present outputs)
            nc.sync.dma_start(kT_o.ap().rearrange("(m p) t -> p m t", p=128), kT_sb)
            v_o_r = v_o.ap().rearrange("(tt p) (h d) -> p tt h d", p=128, h=HPC)
            for tt in range(NTT):
                nc.sync.dma_start(v_o_r[:, tt], v_sb[:, tt, :, 0:64])

            # ---- Phase 2: attention ----
            with tc.tile_pool(name="sps", bufs=3, space="PSUM") as sps, \
                 tc.tile_pool(name="avps", bufs=1, space="PSUM") as avps:
                for m in range(NM):
                    for bb in range(NB):
                        njc = 4 * bb + 4  # causal s-chunk count for this t block
                        ps_y = [avps.tile([65, 512], dt.float32, tag=f"av{h}",
                                          name=f"av_{h}_{m}_{bb}")
                                for h in range(2)]
                        for g in range(njc // 2):
                            for half in range(2):
                                p0 = 64 * half
                                ps_s = sps.tile([128, 2, 512], dt.float32, tag="s")
                                for dj in range(2):
                                    j = 2 * g + dj
                                    nc.tensor.matmul(
                                        ps_s[:, dj, :],
                                        lhsT=kT_sb[p0:p0 + 64, m,
                                                   128 * j:128 * j + 128],
                                        rhs=qT_sb[p0:p0 + 64, m,
                                                  512 * bb:512 * bb + 512],
                                        start=True, stop=True)
                                ex = exp_pool.tile([128, 2, 512], dt.bfloat16,
                                                   tag="ex")
                                nc.scalar.activation(ex, ps_s, AF.Exp, scale=0.125)
                                for dj in range(2):
                                    j = 2 * g + dj
                                    jpos = j - 4 * bb
                                    if jpos >= 0:  # diagonal chunk: causal mask
                                        nc.vector.tensor_tensor(
                                            ex[:, dj, :], ex[:, dj, :],
                                            mk_sb[:, jpos, :], ALU.mult)
                                for dj in range(2):
                                    j = 2 * g + dj
                                    nc.tensor.matmul(
                                        ps_y[half],
                                        lhsT=v_sb[:, j, 2 * m + half, :],
                                        rhs=ex[:, dj, :],
                                        start=(j == 0), stop=(j == njc - 1))
                        for half in range(2):
                            row = small.tile([1, 512], dt.float32, tag="row")
                            nc.vector.tensor_copy(row, ps_y[half][64:65, :])
                            bc = small.tile([64, 512], dt.float32, tag="bc")
                            nc.gpsimd.partition_broadcast(bc, row, channels=64)
                            rec = small.tile([64, 512], dt.float32, tag="rec")
                            nc.vector.reciprocal_approx_fast(out=rec, in_=bc)
                            nc.vector.tensor_tensor(
                                yT_sb[64 * half:64 * half + 64, m,
                                      512 * bb:512 * bb + 512],
                                ps_y[half][0:64, :], rec, ALU.mult)

            # ---- Phase 3: output projection ----
            yp_ap = yp_o.ap().rearrange("(mo p) t -> p mo t", p=128)
            with tc.tile_pool(name="pps", bufs=4, space="PSUM") as pps:
                for mo in range(N_EMBD // 128):
                    for bb in range(NB):
                        ps = pps.tile([128, 512], dt.float32, tag="pp")
                        for kc in range(NM):
                            nc.tensor.matmul(
                                ps,
                                lhsT=wp_sb[:, kc, 128 * mo:128 * mo + 128],
                                rhs=yT_sb[:, kc, 512 * bb:512 * bb + 512],
                                start=(kc == 0), stop=(kc == NM - 1))
                        st = stage.tile([128, 512], dt.bfloat16, tag="st")
                        nc.vector.tensor_copy(st, ps)
                        nc.sync.dma_start(yp_ap[:, mo, 512 * bb:512 * bb + 512], st)

    nc.finalize()
    return nc


def _get_nc():
    if "nc" not in _BUILT:
        _BUILT["nc"] = _build_nc()
    return _BUILT["nc"]


def _make_masks():
    sp = np.arange(128)[:, None]
    tp = np.arange(512)[None, :]
    return np.stack([(tp >= 128 * jpos + sp) for jpos in range(4)],
                    axis=1).astype(BF16)  # [128, 4, 512]


def kernel(x, Wq, bq, Wk, bk, Wv, bv, Wp, bp):
    global LAST_RESULT
    from concourse.bass_utils import run_bass_kernel_spmd

    x = np.asarray(x, F32)
    Wq = np.asarray(Wq, F32); bq = np.asarray(bq, F32)
    Wk = np.asarray(Wk, F32); bk = np.asarray(bk, F32)
    Wv = np.asarray(Wv, F32); bv = np.asarray(bv, F32)
    Wp = np.asarray(Wp, F32); bp = np.asarray(bp, F32)

    nc = _get_nc()
    masks = _make_masks()
    xT = np.ascontiguousarray(x.transpose(0, 2, 1))  # [B, C, T]

    in_maps = []
    for c in range(N_CORES):
        b, g = divmod(c, 2)
        sl = slice(CPC * g, CPC * g + CPC)
        in_maps.append({
            "xT": xT[b].astype(BF16),
            "wq": Wq[:, sl].astype(BF16),
            "wk": Wk[:, sl].astype(BF16),
            "wv": Wv[:, sl].astype(BF16),
            "wp": Wp[sl, :].astype(BF16),
            "bq_r": np.ascontiguousarray(bq[sl].reshape(NM, 128).T),
            "bk_r": np.ascontiguousarray(bk[sl].reshape(NM, 128).T),
            "bv_bc": np.ascontiguousarray(
                np.broadcast_to(bv[sl], (128, CPC))).astype(F32),
            "masks": masks,
        })

    res = run_bass_kernel_spmd(nc, in_maps, core_ids=list(range(N_CORES)))
    LAST_RESULT = res

    y = np.empty((B, T, N_EMBD), F32)
    k = np.empty((B, N_HEAD, T, HEAD_DIM), F32)
    v = np.empty((B, N_HEAD, T, HEAD_DIM), F32)
    for c in range(N_CORES):
        b, g = divmod(c, 2)
        out = res.results[c]
        kT = out["kT_out"].astype(F32)           # [512, T]
        vn = out["v_out"].astype(F32)            # [T, 512]
        for lh in range(HPC):
            h = HPC * g + lh
            k[b, h] = kT[64 * lh:64 * lh + 64, :].T
            v[b, h] = vn[:, 64 * lh:64 * lh + 64]
    for b in range(B):
        ypT = (res.results[2 * b]["ypT_out"].astype(F32)
               + res.results[2 * b + 1]["ypT_out"].astype(F32))  # [C, T]
        y[b] = ypT.T + bp[None, :]

    present = np.stack([k, v])  # [2, B, H, T, D]
    return y, present


# revision 7
# speedup vs baseline: 1.2591x; 1.2591x over previous
"""Causal self-attention (B=4, T=2048, C=1024, H=16, D=64) on 8 TRN2 NeuronCores.

Sharding: core c -> (batch b = c//2, head-group g = c%2 covering heads
8g..8g+8). Data-parallel over B, tensor-parallel over heads. The output
projection is computed per-core over its 512 channels; the two partial
products per batch are summed on the host (the "all-reduce"), where the
projection bias is also added.

Per-core kernel (single SPMD program, per-core data):
  Phase 1: qT = (x Wq + bq)^T and kT likewise, laid out [c'=512, T] (head-major
           on partitions: chunk m holds heads 2m, 2m+1 at partition offsets
           0/64); v in natural layout [T, c'] with an appended ones column per
           head (Vaug, M=65) for the softmax denominator.
  Phase 2: per head pair m, per 512-wide t-block: S^T[s,t] tiles via K=64
           matmuls packed two-heads-per-PE-array (tile_position from
           base_partition 0/64); exp on ScalarE (scale=1/8 folded) straight
           from PSUM into bf16 SBUF; causal masking by multiplying constant
           triangular masks on the diagonal s-chunks; AV matmuls with
           lhsT=Vaug -> unnormalized y^T (rows 0:64) and sumexp (row 64) in
           one accumulation; normalize with partition_broadcast + fast
           reciprocal.
  Phase 3: y_out^T = Wp_g^T @ yT accumulated over the core's 4 channel chunks,
           streamed to DRAM as bf16.

No max-subtraction in softmax: scores are O(1) here (exp is safe in fp32),
and exp(S)/sum(exp(S)) is mathematically identical to jax.nn.softmax.
"""

import numpy as np
import ml_dtypes

BF16 = ml_dtypes.bfloat16
F32 = np.float32

N_EMBD = 1024
N_HEAD = 16
HEAD_DIM = 64
B = 4
T = 2048
N_CORES = 8
HPC = 8          # heads per core
CPC = HPC * HEAD_DIM  # channels per core = 512
NKC = N_EMBD // 128   # contraction chunks over full embed = 8
NM = CPC // 128       # head-pair chunks per core = 4
NB = T // 512         # 512-wide t blocks = 4
NTT = T // 128        # 128-wide t tiles = 16

_BUILT = {}
LAST_RESULT = None  # BassKernelResults of the most recent run (for test harness)


def _build_nc():
    import concourse.bass as bass
    import concourse.mybir as mybir
    import concourse.tile as tile
    from concourse import bacc

    dt = mybir.dt
    AF = mybir.ActivationFunctionType
    ALU = mybir.AluOpType

    nc = bacc.Bacc(trn_type="TRN2", name="csa")

    # ---- DRAM I/O ----
    xT_d = nc.dram_tensor("xT", [N_EMBD, T], dt.bfloat16, kind="ExternalInput")
    wq_d = nc.dram_tensor("wq", [N_EMBD, CPC], dt.bfloat16, kind="ExternalInput")
    wk_d = nc.dram_tensor("wk", [N_EMBD, CPC], dt.bfloat16, kind="ExternalInput")
    wv_d = nc.dram_tensor("wv", [N_EMBD, CPC], dt.bfloat16, kind="ExternalInput")
    wp_d = nc.dram_tensor("wp", [CPC, N_EMBD], dt.bfloat16, kind="ExternalInput")
    bq_d = nc.dram_tensor("bq_r", [128, NM], dt.float32, kind="ExternalInput")
    bk_d = nc.dram_tensor("bk_r", [128, NM], dt.float32, kind="ExternalInput")
    bv_d = nc.dram_tensor("bv_bc", [128, CPC], dt.float32, kind="ExternalInput")
    mk_d = nc.dram_tensor("masks", [128, 4, 512], dt.bfloat16, kind="ExternalInput")

    kT_o = nc.dram_tensor("kT_out", [CPC, T], dt.bfloat16, kind="ExternalOutput")
    v_o = nc.dram_tensor("v_out", [T, CPC], dt.bfloat16, kind="ExternalOutput")
    yp_o = nc.dram_tensor("ypT_out", [N_EMBD, T], dt.bfloat16, kind="ExternalOutput")

    with tile.TileContext(nc) as tc:
        with tc.tile_pool(name="const", bufs=1) as cp, \
             tc.tile_pool(name="ex", bufs=6) as exp_pool, \
             tc.tile_pool(name="stage", bufs=4) as stage, \
             tc.tile_pool(name="small", bufs=4) as small:

            # ---- load constants / inputs (ordered so compute starts early) ----
            wq_sb = cp.tile([128, NKC, CPC], dt.bfloat16, tag="wq")
            nc.sync.dma_start(wq_sb, wq_d.ap().rearrange("(kc p) m -> p kc m", p=128))
            wk_sb = cp.tile([128, NKC, CPC], dt.bfloat16, tag="wk")
            nc.sync.dma_start(wk_sb, wk_d.ap().rearrange("(kc p) m -> p kc m", p=128))
            xT_sb = cp.tile([128, NKC, T], dt.bfloat16, tag="xT")
            xT_r = xT_d.ap().rearrange("(kc p) t -> p kc t", p=128)
            for bb in range(NB):
                sl = slice(512 * bb, 512 * bb + 512)
                nc.sync.dma_start(xT_sb[:, :, sl], xT_r[:, :, sl])
            bq_sb = cp.tile([128, NM], dt.float32, tag="bq")
            nc.sync.dma_start(bq_sb, bq_d.ap())
            bk_sb = cp.tile([128, NM], dt.float32, tag="bk")
            nc.sync.dma_start(bk_sb, bk_d.ap())
            wv_sb = cp.tile([128, NKC, CPC], dt.bfloat16, tag="wv")
            nc.sync.dma_start(wv_sb, wv_d.ap().rearrange("(kc p) m -> p kc m", p=128))
            bv_sb = cp.tile([128, CPC], dt.float32, tag="bv")
            nc.sync.dma_start(bv_sb, bv_d.ap())
            mk_sb = cp.tile([128, 4, 512], dt.bfloat16, tag="mk")
            nc.sync.dma_start(mk_sb, mk_d.ap())
            wp_sb = cp.tile([128, NM, N_EMBD], dt.bfloat16, tag="wp")
            nc.sync.dma_start(wp_sb, wp_d.ap().rearrange("(kc p) m -> p kc m", p=128))

            qT_sb = cp.tile([128, NM, T], dt.bfloat16, tag="qT")
            kT_sb = cp.tile([128, NM, T], dt.bfloat16, tag="kT")
            # Vaug: [t%128, tt, head, 65] (col 64 = ones)
            v_sb = cp.tile([128, NTT, HPC, 65], dt.bfloat16, tag="v")
            yT_sb = cp.tile([128, NM, T], dt.bfloat16, tag="yT")

            nc.gpsimd.memset(v_sb[:, :, :, 64:65], 1.0)

            kT_o_r = kT_o.ap().rearrange("(m p) t -> p m t", p=128)
            v_o_r = v_o.ap().rearrange("(tt p) (h d) -> p tt h d", p=128, h=HPC)

            qkps = tc.tile_pool(name="qkps", bufs=2, space="PSUM")
            qkps_pool = qkps.__enter__()

            def qk_pair(m):
                for (w_sb, bias_sb, out_sb) in ((wq_sb, bq_sb, qT_sb),
                                                (wk_sb, bk_sb, kT_sb)):
                    for bb in range(NB):
                        ps = qkps_pool.tile([128, 512], dt.float32, tag="qk",
                                            name=f"qk_{m}_{bb}")
                        for kc in range(NKC):
                            nc.tensor.matmul(
                                ps,
                                lhsT=w_sb[:, kc, 128 * m:128 * m + 128],
                                rhs=xT_sb[:, kc, 512 * bb:512 * bb + 512],
                                start=(kc == 0), stop=(kc == NKC - 1))
                        nc.vector.tensor_scalar(
                            out=out_sb[:, m, 512 * bb:512 * bb + 512],
                            in0=ps, scalar1=bias_sb[:, m:m + 1], scalar2=None,
                            op0=ALU.add)
                # stream this pair's k to DRAM (present output)
                nc.sync.dma_start(kT_o_r[:, m], kT_sb[:, m])

            def v_all():
                for tt in range(NTT):
                    ps = qkps_pool.tile([128, 512], dt.float32, tag="qk",
                                        name=f"v_{tt}")
                    for kc in range(NKC):
                        nc.tensor.matmul(
                            ps,
                            lhsT=xT_sb[:, kc, 128 * tt:128 * tt + 128],
                            rhs=wv_sb[:, kc, :],
                            start=(kc == 0), stop=(kc == NKC - 1))
                    nc.vector.tensor_tensor(
                        out=v_sb[:, tt, :, 0:64],
                        in0=ps.rearrange("p (h d) -> p h d", h=HPC),
                        in1=bv_sb.rearrange("p (h d) -> p h d", h=HPC),
                        op=ALU.add)
                    nc.sync.dma_start(v_o_r[:, tt], v_sb[:, tt, :, 0:64])

            # ---- Phase 2: attention (per head pair) ----
            sps_cm = tc.tile_pool(name="sps", bufs=2, space="PSUM")
            sps = sps_cm.__enter__()
            avps_cm = tc.tile_pool(name="avps", bufs=1, space="PSUM")
            avps = avps_cm.__enter__()

            def attn_pair(m):
                for bb in range(NB):
                    njc = 4 * bb + 4  # causal s-chunk count for this t block
                    ps_y = [avps.tile([65, 512], dt.float32, tag=f"av{h}",
                                      name=f"av_{h}_{m}_{bb}")
                            for h in range(2)]
                    for g in range(njc // 2):
                        for half in range(2):
                            p0 = 64 * half
                            ps_s = sps.tile([128, 2, 512], dt.float32, tag="s",
                                            name=f"s_{m}_{bb}_{g}_{half}")
                            for dj in range(2):
                                j = 2 * g + dj
                                nc.tensor.matmul(
                                    ps_s[:, dj, :],
                                    lhsT=kT_sb[p0:p0 + 64, m,
                                               128 * j:128 * j + 128],
                                    rhs=qT_sb[p0:p0 + 64, m,
                                              512 * bb:512 * bb + 512],
                                    start=True, stop=True)
                            ex = exp_pool.tile([128, 2, 512], dt.bfloat16,
                                               tag="ex", name=f"ex_{m}_{bb}_{g}_{half}")
                            nc.scalar.activation(ex, ps_s, AF.Exp, scale=0.125)
                            for dj in range(2):
                                j = 2 * g + dj
                                jpos = j - 4 * bb
                                if jpos >= 0:  # diagonal chunk: causal mask
                                    nc.vector.tensor_tensor(
                                        ex[:, dj, :], ex[:, dj, :],
                                        mk_sb[:, jpos, :], ALU.mult)
                            for dj in range(2):
                                j = 2 * g + dj
                                nc.tensor.matmul(
                                    ps_y[half],
                                    lhsT=v_sb[:, j, 2 * m + half, :],
                                    rhs=ex[:, dj, :],
                                    start=(j == 0), stop=(j == njc - 1))
                    for half in range(2):
                        row = small.tile([1, 512], dt.float32, tag="row",
                                         name=f"row_{m}_{bb}_{half}")
                        nc.vector.tensor_copy(row, ps_y[half][64:65, :])
                        bc = small.tile([64, 512], dt.float32, tag="bc",
                                        name=f"bc_{m}_{bb}_{half}")
                        nc.gpsimd.partition_broadcast(bc, row, channels=64)
                        rec = small.tile([64, 512], dt.float32, tag="rec",
                                         name=f"rec_{m}_{bb}_{half}")
                        nc.vector.reciprocal_approx_fast(out=rec, in_=bc)
                        nc.vector.tensor_tensor(
                            yT_sb[64 * half:64 * half + 64, m,
                                  512 * bb:512 * bb + 512],
                            ps_y[half][0:64, :], rec, ALU.mult)

            # interleave: qk0, v, qk1, attn0, qk2, attn1, qk3, attn2, attn3
            qk_pair(0)
            v_all()
            qk_pair(1)
            attn_pair(0)
            qk_pair(2)
            attn_pair(1)
            qk_pair(3)
            attn_pair(2)
            attn_pair(3)

            avps_cm.__exit__(None, None, None)
            sps_cm.__exit__(None, None, None)
            qkps.__exit__(None, None, None)

            # ---- Phase 3: output projection ----
            yp_ap = yp_o.ap().rearrange("(mo p) t -> p mo t", p=128)
            with tc.tile_pool(name="pps", bufs=4, space="PSUM") as pps:
                for mo in range(N_EMBD // 128):
                    for bb in range(NB):
                        ps = pps.tile([128, 512], dt.float32, tag="pp")
                        for kc in range(NM):
                            nc.tensor.matmul(
                                ps,
                                lhsT=wp_sb[:, kc, 128 * mo:128 * mo + 128],
                                rhs=yT_sb[:, kc, 512 * bb:512 * bb + 512],
                                start=(kc == 0), stop=(kc == NM - 1))
                        st = stage.tile([128, 512], dt.bfloat16, tag="st")
                        nc.vector.tensor_copy(st, ps)
                        nc.sync.dma_start(yp_ap[:, mo, 512 * bb:512 * bb + 512], st)

    nc.finalize()
    return nc


def _get_nc():
    if "nc" not in _BUILT:
        _BUILT["nc"] = _build_nc()
    return _BUILT["nc"]


def _make_masks():
    sp = np.arange(128)[:, None]
    tp = np.arange(512)[None, :]
    return np.stack([(tp >= 128 * jpos + sp) for jpos in range(4)],
                    axis=1).astype(BF16)  # [128, 4, 512]


def kernel(x, Wq, bq, Wk, bk, Wv, bv, Wp, bp):
    global LAST_RESULT
    from concourse.bass_utils import run_bass_kernel_spmd

    x = np.asarray(x, F32)
    Wq = np.asarray(Wq, F32); bq = np.asarray(bq, F32)
    Wk = np.asarray(Wk, F32); bk = np.asarray(bk, F32)
    Wv = np.asarray(Wv, F32); bv = np.asarray(bv, F32)
    Wp = np.asarray(Wp, F32); bp = np.asarray(bp, F32)

    nc = _get_nc()
    masks = _make_masks()
    xT = np.ascontiguousarray(x.transpose(0, 2, 1))  # [B, C, T]

    in_maps = []
    for c in range(N_CORES):
        b, g = divmod(c, 2)
        sl = slice(CPC * g, CPC * g + CPC)
        in_maps.append({
            "xT": xT[b].astype(BF16),
            "wq": Wq[:, sl].astype(BF16),
            "wk": Wk[:, sl].astype(BF16),
            "wv": Wv[:, sl].astype(BF16),
            "wp": Wp[sl, :].astype(BF16),
            "bq_r": np.ascontiguousarray(bq[sl].reshape(NM, 128).T),
            "bk_r": np.ascontiguousarray(bk[sl].reshape(NM, 128).T),
            "bv_bc": np.ascontiguousarray(
                np.broadcast_to(bv[sl], (128, CPC))).astype(F32),
            "masks": masks,
        })

    res = run_bass_kernel_spmd(nc, in_maps, core_ids=list(range(N_CORES)))
    LAST_RESULT = res

    y = np.empty((B, T, N_EMBD), F32)
    k = np.empty((B, N_HEAD, T, HEAD_DIM), F32)
    v = np.empty((B, N_HEAD, T, HEAD_DIM), F32)
    for c in range(N_CORES):
        b, g = divmod(c, 2)
        out = res.results[c]
        kT = out["kT_out"].astype(F32)           # [512, T]
        vn = out["v_out"].astype(F32)            # [T, 512]
        for lh in range(HPC):
            h = HPC * g + lh
            k[b, h] = kT[64 * lh:64 * lh + 64, :].T
            v[b, h] = vn[:, 64 * lh:64 * lh + 64]
    for b in range(B):
        ypT = (res.results[2 * b]["ypT_out"].astype(F32)
               + res.results[2 * b + 1]["ypT_out"].astype(F32))  # [C, T]
        y[b] = ypT.T + bp[None, :]

    present = np.stack([k, v])  # [2, B, H, T, D]
    return y, present
